# revision 26
# baseline (speedup 1.0000x reference)
"""CrossAttention TRN2 Bass kernel — 8-core data-parallel (batch x query-half).

Sharding: core c -> batch b=c//2, query rows [(c%2)*1024, (c%2+1)*1024).
Each core computes its 1024 output rows end-to-end (kv recomputed per
core-pair; no collectives). Host pre-transposes activations so every
matmul operand is contraction-major in DRAM, and pre-casts x/y/weights
to bf16 (halves DMA + SBUF; matmuls accumulate fp32 in PSUM).

kv compaction: masked kv positions are gathered out on the host (pad to
M2C=544 rows of zeros; actual max kept is 515 for the seed-0 mask).
Zero k rows give exp(0)=1 at pads, but the ones-column appended to v
carries the keep flag, so pads contribute exactly 0 to both the
attention numerator and the softmax denominator.

The 32-row tail m-chunk of the scores matmul is packed head-pairwise
through a block-diagonal zero-padded kT5 tile so it rides the full
128-partition contraction. Scores are built transposed (S^T[m,n]) so
the softmax denominator falls out of the PE ones-column trick;
normalization runs recip(DVE) -> partition broadcast(gpsimd) ->
mul(DVE) in fp32.
"""

import sys

sys.path.insert(0, "/opt/trn_rl_repo")

from contextlib import ExitStack

import ml_dtypes
import numpy as np

import concourse.bass as bass
import concourse.tile as tile
from concourse import bacc, mybir
from concourse.bass_utils import run_bass_kernel_spmd

B, N, N2 = 4, 2048, 1024
DIM, H, HD = 1024, 16, 64
SCALE = HD ** -0.5
P = 128
R = 1024          # query rows per core
NCORES = 8
KO = DIM // P     # 8 contraction chunks
F32 = mybir.dt.float32
BF16 = mybir.dt.bfloat16
NP_BF16 = ml_dtypes.bfloat16

M2C = 544         # compacted kv length; 4x128 + 32 tail

TRACE = False


def build_kernel(m2c=M2C):
    fo = m2c // P         # full 128-wide m-chunks
    rm = m2c % P          # tail chunk width (0 or 32)
    mo_n = fo + (1 if rm else 0)
    assert rm in (0, 32), rm
    kchunk = 272 if m2c % 272 == 0 else 320  # k-proj psum free chunk
    kv_chunks = [(i, min(kchunk, m2c - i)) for i in range(0, m2c, kchunk)]

    nc = bacc.Bacc("TRN2", target_bir_lowering=False, debug=False,
                   num_devices=NCORES)
    xT = nc.dram_tensor("xT", [DIM, R], BF16, kind="ExternalInput").ap()
    yT = nc.dram_tensor("yT", [DIM, m2c], BF16, kind="ExternalInput").ap()
    wq = nc.dram_tensor("wq", [DIM, DIM], BF16, kind="ExternalInput").ap()
    wk = nc.dram_tensor("wk", [DIM, DIM], BF16, kind="ExternalInput").ap()
    wv = nc.dram_tensor("wv", [DIM, DIM], BF16, kind="ExternalInput").ap()
    wp = nc.dram_tensor("wp", [DIM, DIM], BF16, kind="ExternalInput").ap()
    keepc = nc.dram_tensor("keepc", [m2c], F32, kind="ExternalInput").ap()
    bp = nc.dram_tensor("bp", [DIM], F32, kind="ExternalInput").ap()
    out = nc.dram_tensor("out", [DIM, R], F32, kind="ExternalOutput").ap()

    with tile.TileContext(nc, pool_alloc_mode="queue") as tc, ExitStack() as ctx:
        persist = ctx.enter_context(tc.tile_pool(name="persist", bufs=1))
        qT = persist.tile([P, KO, R], BF16)          # q^T, c-major
        kT = persist.tile([P, KO, m2c], BF16)        # k^T, c-major
        vS = persist.tile([P, fo, H * 65], BF16)     # v[m,c] + keep col / head
        attnT = persist.tile([P, KO, R], BF16)       # attn out^T, c-major
        wp_r = persist.tile([P, KO, DIM], BF16)      # Wproj resident
        kc = persist.tile([P, fo + 1 if rm else fo], F32)  # keep col, m-major
        bT = persist.tile([P, KO], F32)
        if rm:
            kT5 = persist.tile([P, KO, 2 * rm], BF16)  # head-pair packed tail
            # tail v rows duplicated at both partition offsets so the tail
            # attn@v lhsT base partition matches exp5's per-head rows
            vS5 = persist.tile([2 * rm, H * 65], BF16)

        vH = vS.rearrange("p mo (h s) -> p mo h s", s=65)
        if rm:
            vH5 = vS5.rearrange("p (h s) -> p h s", s=65)

        # ---- Phase A: projections, with early attention-score tiles
        # interleaved into the k-projection so ACT exp work (the phase-B
        # pacing engine) starts ~40us before v-proj finishes.
        groups = [(0, 2), (2, 2)] if fo == 4 else [(0, 3), (3, 2)]
        EARLY_CO = 3 if rm else 0     # tiles (nn, co<EARLY_CO) hoisted

        pbe_early = ctx.enter_context(tc.tile_pool(name="pBearly", bufs=2 * EARLY_CO or 1))
        pbe = None   # set when phase B opens

        def emit_scores(nn, co, pool, spool, s5pool, one_bank):
            nsl = slice(nn * 512, (nn + 1) * 512)
            state = {"nn": nn, "co": co}
            if rm:
                sp5 = s5pool.tile([P, 512], F32, tag="sp5")
                nc.tensor.matmul(sp5[0:2 * rm], kT5[:, co],
                                 qT[:, co, nsl], start=True, stop=True)
                exp5 = pool.tile([2 * rm, 512], BF16, tag="exp5")
                nc.scalar.activation(
                    exp5[:], sp5[0:2 * rm],
                    mybir.ActivationFunctionType.Exp,
                    scale=float(SCALE))
                state["exp5"] = exp5
            for hh in (2 * co, 2 * co + 1):
                pq = (hh % 2) * 64
                expS = pool.tile([P, fo, 512], BF16, tag=f"expS{hh % 2}")
                state[f"expS{hh % 2}"] = expS
                if one_bank:
                    for mo in range(fo):
                        sps = spool.tile([P, 512], F32, tag="sps")
                        nc.tensor.matmul(
                            sps[:], kT[pq:pq + 64, co, mo * P:(mo + 1) * P],
                            qT[pq:pq + 64, co, nsl], start=True, stop=True)
                        nc.scalar.activation(
                            expS[:, mo], sps[:],
                            mybir.ActivationFunctionType.Exp,
                            scale=float(SCALE))
                else:
                    for g0, gn in groups:
                        sps = spool.tile([P, 2, 512], F32, tag="sps")
                        for mo in range(g0, g0 + gn):
                            nc.tensor.matmul(
                                sps[:, mo - g0],
                                kT[pq:pq + 64, co, mo * P:(mo + 1) * P],
                                qT[pq:pq + 64, co, nsl],
                                start=True, stop=True)
                        nc.scalar.activation(
                            expS[:, g0:g0 + gn], sps[:, :gn],
                            mybir.ActivationFunctionType.Exp,
                            scale=float(SCALE))
            return state

        def emit_av(state, pool, opool):
            nn, co = state["nn"], state["co"]
            nsl = slice(nn * 512, (nn + 1) * 512)
            for hh in (2 * co, 2 * co + 1):
                pq = (hh % 2) * 64
                expS = state[f"expS{hh % 2}"]
                ops = opool.tile([P, 512], F32, tag="ops")
                for mo in range(fo):
                    nc.tensor.matmul(
                        ops[0:65], vS[:, mo, hh * 65:(hh + 1) * 65],
                        expS[:, mo],
                        start=(mo == 0),
                        stop=(mo == mo_n - 1 and not rm))
                if rm:
                    off = (hh % 2) * rm
                    nc.tensor.matmul(
                        ops[0:65], vS5[off:off + rm, hh * 65:(hh + 1) * 65],
                        state["exp5"][off:off + rm],
                        start=False, stop=True)
                rec = pool.tile([1, 512], F32, tag="rec")
                nc.vector.reciprocal(rec[:], ops[64:65])
                bc = pool.tile([64, 512], F32, tag="bc")
                nc.gpsimd.partition_broadcast(bc[:], rec[:])
                nc.vector.tensor_mul(
                    attnT[pq:pq + 64, co, nsl], ops[0:64], bc[:])

        early_states = []
        with tc.tile_pool(name="pAq", bufs=1) as paq, \
             tc.tile_pool(name="pAkv", bufs=1) as pakv, \
             tc.tile_pool(name="psA", bufs=2, space="PSUM") as psa, \
             tc.tile_pool(name="psSe", bufs=2, space="PSUM") as psse, \
             tc.tile_pool(name="ps5e", bufs=2, space="PSUM") as ps5e:
            xT_r = paq.tile([P, KO, R], BF16)
            wq_r = paq.tile([P, KO, DIM], BF16)
            yT_r = pakv.tile([P, KO, m2c], BF16)
            wk_r = pakv.tile([P, KO, DIM], BF16)
            wv_r = pakv.tile([P, KO, DIM], BF16)
            # DMA order = consumption order: k-proj inputs (smallest) first so
            # PE starts earliest; its 14.5us covers the x/wq load behind it.
            for ko in range(KO):
                nc.sync.dma_start(yT_r[:, ko],
                                  yT.rearrange("(ko p) f -> p ko f", p=P)[:, ko])
                nc.sync.dma_start(wk_r[:, ko],
                                  wk.rearrange("(ko p) c -> p ko c", p=P)[:, ko])
            for ko in range(KO):
                nc.sync.dma_start(xT_r[:, ko],
                                  xT.rearrange("(ko p) f -> p ko f", p=P)[:, ko])
                nc.sync.dma_start(wq_r[:, ko],
                                  wq.rearrange("(ko p) c -> p ko c", p=P)[:, ko])
            for ko in range(KO):
                nc.sync.dma_start(wv_r[:, ko],
                                  wv.rearrange("(ko p) c -> p ko c", p=P)[:, ko])
            # small loads (needed from phase B on; keep off the DMA fast path)
            nc.sync.dma_start(kc[:, :fo],
                              keepc[:fo * P].rearrange("(mo p) -> p mo", p=P))
            if rm:
                nc.sync.dma_start(
                    kc[:rm, fo:fo + 1],
                    keepc[fo * P:].rearrange("(mo p) -> p mo", p=rm))
            nc.sync.dma_start(bT[:], bp.rearrange("(o p) -> p o", p=P))
            for ko in range(KO):
                nc.sync.dma_start(wp_r[:, ko],
                                  wp.rearrange("(ko p) c -> p ko c", p=P)[:, ko])

            # k = y @ Wk (kT c-major)
            if rm:
                nc.vector.memset(kT5[:], 0.0)
            for co in range(KO):
                for m0, mw in kv_chunks:
                    ps = psa.tile([P, kchunk], F32, tag="pskv")
                    # rotate accumulation order so chain co starts on the
                    # ko-chunk whose DMA lands as the chain starts
                    for i in range(KO):
                        ko = (co + i) % KO
                        nc.tensor.matmul(
                            ps[:, :mw], wk_r[:, ko, co * P:(co + 1) * P],
                            yT_r[:, ko, m0:m0 + mw],
                            start=(i == 0), stop=(i == KO - 1))
                    nc.vector.tensor_copy(kT[:, co, m0:m0 + mw], ps[:, :mw])
                if rm:
                    # head-pair packed tail chunk of k^T (block-diag zero-pad)
                    nc.vector.tensor_copy(kT5[0:64, co, 0:rm],
                                          kT[0:64, co, fo * P:])
                    nc.vector.tensor_copy(kT5[64:128, co, rm:2 * rm],
                                          kT[64:128, co, fo * P:])

            # q = x @ Wq, early score tiles right after their qT column lands
            for co in range(KO):
                for nn in range(2):
                    ps = psa.tile([P, 512], F32, tag="psq")
                    for ko in range(KO):
                        nc.tensor.matmul(
                            ps[:], wq_r[:, ko, co * P:(co + 1) * P],
                            xT_r[:, ko, nn * 512:(nn + 1) * 512],
                            start=(ko == 0), stop=(ko == KO - 1))
                    nc.vector.tensor_copy(qT[:, co, nn * 512:(nn + 1) * 512],
                                          ps[:])
                if co < EARLY_CO:
                    for nn in range(2):
                        early_states.append(emit_scores(
                            nn, co, pbe_early, psse, ps5e, one_bank=True))

            # v = y @ Wv (vS m-major, 65-wide per head with keep col)
            for c4 in range(4):  # 256-wide v column chunks (4 heads)
                for mo in range(mo_n):
                    mp = rm if (rm and mo == fo) else P
                    ps = psa.tile([P, 256], F32, tag="pskv")
                    for ko in range(KO):
                        nc.tensor.matmul(
                            ps[:mp], yT_r[:, ko, mo * P:mo * P + mp],
                            wv_r[:, ko, c4 * 256:(c4 + 1) * 256],
                            start=(ko == 0), stop=(ko == KO - 1))
                    if rm and mo == fo:
                        nc.vector.tensor_copy(
                            vH5[:rm, c4 * 4:(c4 + 1) * 4, 0:64],
                            ps[:rm].rearrange("p (h d) -> p h d", d=64))
                    else:
                        nc.vector.tensor_copy(
                            vH[:, mo, c4 * 4:(c4 + 1) * 4, 0:64],
                            ps[:].rearrange("p (h d) -> p h d", d=64))
            # keep cols (emitted after v-proj copies so the DVE queue is not
            # blocked early waiting for the late kc DMA)
            for mo in range(fo):
                nc.vector.tensor_copy(vH[:, mo, :, 64],
                                      kc[:, mo:mo + 1].to_broadcast([P, H]))
            if rm:
                nc.vector.tensor_copy(vH5[:rm, :, 64],
                                      kc[:rm, fo:fo + 1].to_broadcast([rm, H]))
                # duplicate tail v (+keep col) to partition offset rm so the
                # odd head's tail attn@v sees matching base partitions
                nc.sync.dma_start(vS5[rm:2 * rm, :], vS5[0:rm, :])

        # ---- Phase B: late tiles' scores interleaved with attn@v of all
        # tiles (av(i) runs several tiles behind its exp, so PE never waits).
        # avs are consumed in (nn, co) order so phase C's nn=0 inputs are
        # normalized well before the last av retires.
        late = [(nn, co) for nn in range(2) for co in range(EARLY_CO, KO)]
        states = {(s["nn"], s["co"]): s for s in early_states}
        av_order = [(nn, co) for nn in range(2) for co in range(KO)]
        with tc.tile_pool(name="pBe", bufs=5) as pbe, \
             tc.tile_pool(name="outp", bufs=3) as outp, \
             tc.tile_pool(name="psS", bufs=2, space="PSUM") as pss, \
             tc.tile_pool(name="ps5", bufs=1, space="PSUM") as ps5p, \
             tc.tile_pool(name="psO", bufs=3, space="PSUM") as pso, \
             tc.tile_pool(name="psC", bufs=2, space="PSUM") as psc:

            def emit_proj(nn, c2o):
                ps = psc.tile([P, 512], F32, tag="psc")
                for co in range(KO):
                    nc.tensor.matmul(
                        ps[:], wp_r[:, co, c2o * P:(c2o + 1) * P],
                        attnT[:, co, nn * 512:(nn + 1) * 512],
                        start=(co == 0), stop=(co == KO - 1))
                osb = outp.tile([P, 512], F32, tag="osb")
                nc.vector.tensor_scalar_add(osb[:], ps[:], bT[:, c2o:c2o + 1])
                nc.sync.dma_start(
                    out[c2o * P:(c2o + 1) * P, nn * 512:(nn + 1) * 512],
                    osb[:])

            avi = 0
            for nn, co in late:
                states[(nn, co)] = emit_scores(nn, co, pbe, pss, ps5p,
                                               one_bank=True)
                emit_av(states[av_order[avi]], pbe, pso)
                avi += 1
            # av tail interleaved with phase-C nn=0 chains: proj work hides
            # the DVE normalize chain that would otherwise stall psO reuse
            c_chains = [(nn, c2o) for nn in range(2) for c2o in range(KO)]
            ci = 0
            while avi < len(av_order):
                emit_av(states[av_order[avi]], pbe, pso)
                avi += 1
                if ci < 6:
                    emit_proj(*c_chains[ci])
                    ci += 1
            while ci < len(c_chains):
                emit_proj(*c_chains[ci])
                ci += 1

    nc.finalize()
    return nc


_NC = {}


def kernel(x, y, pad_mask, Wq, Wkv, Wproj, bproj):
    x = np.asarray(x, dtype=np.float32)
    y = np.asarray(y, dtype=np.float32)
    pad_mask = np.asarray(pad_mask)
    Wkv = np.asarray(Wkv, dtype=np.float32)
    bproj = np.asarray(bproj, dtype=np.float32)

    Wq16 = np.ascontiguousarray(np.asarray(Wq).astype(NP_BF16))
    Wk16 = np.ascontiguousarray(Wkv[:, :DIM].astype(NP_BF16))
    Wv16 = np.ascontiguousarray(Wkv[:, DIM:].astype(NP_BF16))
    Wp16 = np.ascontiguousarray(np.asarray(Wproj).astype(NP_BF16))

    # compact kv: gather kept rows per batch, pad with zeros to m2c
    keep_idx = [np.nonzero(pad_mask[b] != 0)[0] for b in range(B)]
    max_kept = max(len(i) for i in keep_idx)
    m2c = next(m for m in (M2C, 640, N2) if max_kept <= m)
    yc = np.zeros((B, m2c, DIM), dtype=np.float32)
    keepc = np.zeros((B, m2c), dtype=np.float32)
    for b in range(B):
        k = len(keep_idx[b])
        yc[b, :k] = y[b][keep_idx[b]]
        keepc[b, :k] = 1.0

    in_maps = []
    for c in range(NCORES):
        b, half = c // 2, c % 2
        in_maps.append({
            "xT": np.ascontiguousarray(
                x[b, half * R:(half + 1) * R, :].T.astype(NP_BF16)),
            "yT": np.ascontiguousarray(yc[b].T.astype(NP_BF16)),
            "wq": Wq16, "wk": Wk16, "wv": Wv16, "wp": Wp16,
            "keepc": keepc[b],
            "bp": bproj,
        })

    if m2c not in _NC:
        _NC[m2c] = build_kernel(m2c)

    res = run_bass_kernel_spmd(_NC[m2c], in_maps, core_ids=list(range(NCORES)),
                               trace=TRACE)
    if TRACE:
        kernel.last_results = res

    full = np.empty((B, N, DIM), dtype=np.float32)
    for c in range(NCORES):
        b, half = c // 2, c % 2
        full[b, half * R:(half + 1) * R, :] = res.results[c]["out"].T
    return full
